# revision 1
# baseline (speedup 1.0000x reference)
"""DWARF attention Trainium2 Bass kernel.

Sharding: 8 cores = 4 batches x 2 head-halves (8 local heads each).
Per-core dataflow (feature-major = [feature rows, token cols]):
  P1 proj:  q/k/v/gate = W^T.T @ xT on PE (f32r), ACT evictions w/ bias+sigmoid
  P2 E:     E = exp(q_offset + prior) token-major, SE row-sums
  P3 qk:    per (pair, offset): prod = q2*k2_shift (DVE) -> pair-sum matmul (PE)
            -> ACT evict [2,N] -> DMA-assemble qk_T [88,N] x2
  P4 tm:    PE-transpose qk_T -> qk_tm token-major
  P5 feat:  gpsimd tap-gathers, feat=elu(qk+b)+1, A=E*coef*feat/z (DVE/ACT)
  P6 A_T:   PE-transpose A_tm -> A_stage -> DMA-pack A rows 32-aligned
  P7 AV:    PE rank1 A_exp psum -> DVE mul with v2 -> PE identity accumulate
  P8 out:   gg = out_fm*gate; y_fm = Wout^T.T @ gg (PE) -> DRAM
Host: shard, pre-transpose weights, build tap tables, reduce head-halves.
"""
from contextlib import ExitStack

import numpy as np

import concourse.bass as bass
import concourse.mybir as mybir
import concourse.tile as tile
from concourse import bacc
from concourse.bass_utils import run_bass_kernel_spmd
from concourse.masks import make_identity

F32 = mybir.dt.float32
F32R = mybir.dt.float32r
U16 = mybir.dt.uint16
AF = mybir.ActivationFunctionType
ALU = mybir.AluOpType
AX = mybir.AxisListType

B, N, D, H = 4, 2048, 1024, 16
HD = 64
NS = 11
HL = 8
NPAIR = 4
D4 = [0.4829629131445341, 0.8365163037378079, 0.2241438680420134, -0.1294095225512604]

TAPS = []
for _j in range(NS):
    _d = 1 << _j
    for _tau in range(4):
        _off = _d * _tau
        if _off != 0 and _off >= N:
            continue
        TAPS.append((_j, _tau, _off, 4 * _j + _tau))
NTAP = len(TAPS)            # 42
NTB = 44                    # full (j, tau) grid; invalid slots get coef 0
OFFSETS = sorted({t[2] for t in TAPS})
NOFF = len(OFFSETS)         # 22
OFF_IDX = {o: i for i, o in enumerate(OFFSETS)}
NCH = N // 128
NNB = N // 512

_KERNEL_CACHE = {}
PHASE_LIMIT = 9  # debug: build only phases < this


def build_kernel():
    nc = bacc.Bacc("TRN2", target_bir_lowering=False, debug=False, num_devices=8)

    xT = nc.dram_tensor("xT", [D, N], F32R, kind="ExternalInput")
    wT = nc.dram_tensor("wT", [D, 4 * HL * HD], F32R, kind="ExternalInput")
    woT = nc.dram_tensor("woT", [HL * HD, D], F32R, kind="ExternalInput")
    wqs = nc.dram_tensor("wqs", [128, 2 * NS], F32R, kind="ExternalInput")
    prior_c = nc.dram_tensor("prior_c", [128, HL * NS], F32, kind="ExternalInput")
    bias_c = nc.dram_tensor("bias_c", [128, HL * NTB], F32, kind="ExternalInput")
    coef_c = nc.dram_tensor("coef_c", [128, HL * NTB], F32, kind="ExternalInput")
    bqkv_c = nc.dram_tensor("bqkv_c", [128, 12], F32, kind="ExternalInput")
    bgate_c = nc.dram_tensor("bgate_c", [128, NPAIR], F32, kind="ExternalInput")
    bp_c = nc.dram_tensor("bp_c", [128, HL], F32, kind="ExternalInput")
    opair_c = nc.dram_tensor("opair_c", [128, 128], F32R, kind="ExternalInput")

    gate_d = nc.dram_tensor("gate_d", [NPAIR, 128, N], F32)
    v_d = nc.dram_tensor("v_d", [NPAIR, 128, N], F32R)
    y_fm = nc.dram_tensor("y_fm", [D, N], F32, kind="ExternalOutput")

    with tile.TileContext(nc) as tc, ExitStack() as S:
        # ---- persistent constants (small) ----
        const = S.enter_context(tc.tile_pool(name="const", bufs=1))
        ident_f = const.tile([128, 128], F32)
        make_identity(nc, ident_f)
        ident = const.tile([128, 128], F32R)
        nc.vector.tensor_copy(ident[:], ident_f[:])
        ones_f = const.tile([128, 2], F32)
        nc.vector.memset(ones_f[:], 0.0)
        nc.vector.memset(ones_f[0:64, 0:1], 1.0)
        nc.vector.memset(ones_f[64:128, 1:2], 1.0)
        ones2 = const.tile([128, 2], F32R)
        nc.vector.tensor_copy(ones2[:], ones_f[:])
        opair = const.tile([128, 128], F32R)
        nc.sync.dma_start(opair[:], opair_c[:])
        wqs_r = const.tile([128, 2 * NS], F32R)
        nc.sync.dma_start(wqs_r[:], wqs[:])
        prior_t = const.tile([128, HL * NS], F32)
        nc.sync.dma_start(prior_t[:], prior_c[:])
        bias_t = const.tile([128, HL * NTB], F32)
        nc.sync.dma_start(bias_t[:], bias_c[:])
        coef_t = const.tile([128, HL * NTB], F32)
        nc.sync.dma_start(coef_t[:], coef_c[:])
        bqkv_t = const.tile([128, 12], F32)
        nc.sync.dma_start(bqkv_t[:], bqkv_c[:])
        bgate_t = const.tile([128, NPAIR], F32)
        nc.sync.dma_start(bgate_t[:], bgate_c[:])
        bp_t = const.tile([128, HL], F32)
        nc.sync.dma_start(bp_t[:], bp_c[:])

        # E/qk_tm pool: P2..P5
        S_e = ExitStack()
        ep = S_e.enter_context(tc.tile_pool(name="ep", bufs=1))
        E_tm = ep.tile([128, NCH * HL * NS], F32)
        SE_tm = ep.tile([128, NCH * HL], F32)

        # ======== P1-P4 fused scope ========
        with tc.tile_pool(name="p1x", bufs=1) as p1x, \
             tc.tile_pool(name="p1", bufs=2) as p1, \
             tc.tile_pool(name="p1g", bufs=2) as p1g, \
             tc.tile_pool(name="p3", bufs=2) as p3, \
             tc.tile_pool(name="p3s", bufs=2) as p3s, \
             tc.tile_pool(name="qkTp", bufs=1, side="right") as qkTp:
            qk_T = [qkTp.tile([88, N], F32, tag=f"qkT{t}", name=f"qkT{t}")
                    for t in range(2)]
            S_qk = ExitStack()
            qk2p = S_qk.enter_context(tc.tile_pool(name="qk2", bufs=1, side="right"))
            q2 = [qk2p.tile([128, N], F32R, tag=f"q2_{p}", name=f"q2_{p}")
                  for p in range(NPAIR)]
            k2 = [qk2p.tile([128, N], F32R, tag=f"k2_{p}", name=f"k2_{p}")
                  for p in range(NPAIR)]
            xr = []
            for kc in range(D // 128):
                xrt = p1x.tile([128, N], F32R, tag=f"xr{kc}", name=f"xr{kc}")
                nc.sync.dma_start(xrt[:], xT[128 * kc:128 * (kc + 1), :])
                xr.append(xrt)
            p1ps_cm = tc.tile_pool(name="p1ps", bufs=4, space="PSUM")
            p1ps = p1ps_cm.__enter__()
            for fc in range(16):
                wr = p1.tile([128, 8 * 128], F32R, tag="wr")
                nc.sync.dma_start(
                    wr[:].rearrange("p (a m) -> p a m", a=8),
                    wT[:, 128 * fc:128 * (fc + 1)]
                    .rearrange("(a p) m -> p a m", p=128))
                sect, pair = fc // 4, fc % 4
                for nb in range(NNB):
                    ps = p1ps.tile([128, 512], F32, tag="proj")
                    for kc in range(D // 128):
                        nc.tensor.matmul(
                            ps[:], wr[:, 128 * kc:128 * (kc + 1)],
                            xr[kc][:, 512 * nb:512 * (nb + 1)],
                            start=(kc == 0), stop=(kc == 7))
                    sl = slice(512 * nb, 512 * (nb + 1))
                    if sect == 0:
                        nc.scalar.activation(q2[pair][:, sl], ps[:],
                                             AF.Identity, bias=bqkv_t[:, fc:fc + 1])
                    elif sect == 1:
                        nc.scalar.activation(k2[pair][:, sl], ps[:],
                                             AF.Identity, bias=bqkv_t[:, fc:fc + 1])
                    elif sect == 2:
                        vt = p1g.tile([128, 512], F32R, tag="vt")
                        nc.scalar.activation(vt[:], ps[:],
                                             AF.Identity, bias=bqkv_t[:, fc:fc + 1])
                        nc.sync.dma_start(v_d[pair, :, sl], vt[:])
                    else:
                        gt = p1g.tile([128, 512], F32, tag="gt")
                        nc.scalar.activation(gt[:], ps[:], AF.Sigmoid,
                                             bias=bgate_t[:, pair:pair + 1])
                        nc.sync.dma_start(gate_d[pair, :, sl], gt[:])

            p1ps_cm.__exit__(None, None, None)
            # ---- P2: E = exp(q_offset + prior), SE ----
            p2ps_cm = tc.tile_pool(name="p2ps", bufs=2, space="PSUM")
            p2ps = p2ps_cm.__enter__()
            for c in range(NCH):
                ps = p2ps.tile([128, HL * NS], F32, tag="qs")
                for p in range(NPAIR):
                    nc.tensor.matmul(
                        ps[:, 2 * NS * p:2 * NS * (p + 1)],
                        q2[p][:, 128 * c:128 * (c + 1)], wqs_r[:],
                        start=True, stop=True)
                et = p1g.tile([128, HL * NS], F32, tag="et")
                nc.vector.tensor_tensor(out=et[:], in0=ps[:], in1=prior_t[:],
                                        op=ALU.add)
                nc.scalar.activation(E_tm[:, HL * NS * c:HL * NS * (c + 1)],
                                     et[:], AF.Exp)
            nc.vector.tensor_reduce(
                SE_tm[:].rearrange("p (c h) -> p c h", h=HL).unsqueeze(-1),
                E_tm[:].rearrange("p (c h s) -> p c h s", h=HL, s=NS),
                axis=AX.X, op=ALU.add)

            p2ps_cm.__exit__(None, None, None)
            # ---- P3: qk products -> pair-sum -> qk_T assembly ----
            p3ps_cm = tc.tile_pool(name="p3ps", bufs=2, space="PSUM")
            p3ps = p3ps_cm.__enter__()
            for p in range(NPAIR):
                for oi, off in enumerate(OFFSETS):
                    prod = p3.tile([128, N], F32R, tag="prod")
                    if off > 0:
                        nc.gpsimd.memset(prod[:, 0:off].bitcast(F32), 0.0)
                    prod_eng = nc.gpsimd if oi % 3 == 2 else nc.vector
                    prod_eng.tensor_tensor(
                        out=prod[:, off:N], in0=q2[p][:, off:N],
                        in1=k2[p][:, 0:N - off], op=ALU.mult)
                    ps = p3ps.tile([2, N], F32, tag="qkps")
                    for nb in range(NNB):
                        nc.tensor.matmul(
                            ps[:, 512 * nb:512 * (nb + 1)], ones2[:],
                            prod[:, 512 * nb:512 * (nb + 1)],
                            start=True, stop=True)
                    tl = qk_T[p // 2]
                    r0 = (2 * (p % 2)) * NOFF + oi
                    for hf in range(2):
                        st = p3s.tile([2, 1024], F32, tag="stage")
                        hs = slice(1024 * hf, 1024 * (hf + 1))
                        nc.scalar.copy(st[:], ps[:, hs])
                        nc.sync.dma_start(tl[r0:r0 + 1, hs], st[0:1, :])
                        nc.sync.dma_start(tl[r0 + NOFF:r0 + NOFF + 1, hs],
                                          st[1:2, :])
            p3ps_cm.__exit__(None, None, None)
            # q2/k2 released after P3
            S_qk.close()
            qk_tm = ep.tile([128, NCH * 2 * 88], F32, name="qk_tm")

            # ---- P4: transpose qk_T -> token-major ----
            with tc.tile_pool(name="p4ps", bufs=4, space="PSUM") as p4ps:
                for c in range(NCH):
                    for t in range(2):
                        ps = p4ps.tile([128, 88], F32, tag="tp")
                        nc.tensor.transpose(
                            ps[:], qk_T[t][:, 128 * c:128 * (c + 1)],
                            ident_f[0:88, 0:88])
                        nc.scalar.copy(
                            qk_tm[:, 176 * c + 88 * t:176 * c + 88 * (t + 1)],
                            ps[:])

        if PHASE_LIMIT < 5:
            S_e.close()
            nc.compile()
            return nc
        # ======== P5: feat chain ========
        S_as = ExitStack()
        arow = S_as.enter_context(tc.tile_pool(name="arow", bufs=1, side="right"))
        A_stage = [arow.tile([88, N], F32R, tag=f"ast{t}", name=f"ast{t}")
                   for t in range(2)]
        with tc.tile_pool(name="p5", bufs=1) as p5, \
             tc.tile_pool(name="p6ps", bufs=4, space="PSUM") as p6ps:
            W = NCH * HL * NTB          # 5632
            CH = NCH * HL               # 128 (c,h) groups
            # views: qk_tm cols = (ch, omega[22]) ; omega split (e[11], two[2])
            qk8 = qk_tm[:].rearrange("p (ch o) -> p ch o", o=NOFF)
            qk9 = qk_tm[:].rearrange("p (ch e two) -> p ch e two", e=NS, two=2)
            E8 = E_tm[:].rearrange("p (ch s) -> p ch s", s=NS)

            # ---- expand taps: y_t [128, (ch, j, tau)] ----
            y_t = p5.tile([128, W], F32, tag="y")
            y7 = y_t[:].rearrange("p (ch j f) -> p ch j f", j=NS, f=4)
            # tau=0: qk_0 for all j
            nc.gpsimd.tensor_copy(
                y7[:, :, :, 0:1],
                qk8[:, :, 0:1].unsqueeze(2).broadcast_to([128, CH, NS, 1]))
            # tau=1: j=0 -> oi1 ; j=1 -> oi2 ; j>=2 -> oi=2j
            nc.gpsimd.tensor_copy(y7[:, :, 0:1, 1:2],
                                  qk8[:, :, 1:2].unsqueeze(2))
            nc.gpsimd.tensor_copy(y7[:, :, 1:2, 1:2],
                                  qk8[:, :, 2:3].unsqueeze(2))
            nc.gpsimd.tensor_copy(y7[:, :, 2:11, 1:2],
                                  qk9[:, :, 2:11, 0:1])
            # tau=2: j=0 -> oi2 ; j=1..9 -> oi=2j+2 ; j=10 invalid (dup j=9)
            nc.gpsimd.tensor_copy(y7[:, :, 0:1, 2:3],
                                  qk8[:, :, 2:3].unsqueeze(2))
            nc.gpsimd.tensor_copy(y7[:, :, 1:10, 2:3],
                                  qk9[:, :, 2:11, 0:1])
            nc.gpsimd.tensor_copy(y7[:, :, 10:11, 2:3],
                                  qk9[:, :, 10:11, 0:1])
            # tau=3: j=0..9 -> two=1, e=j+1 ; j=10 invalid (dup)
            nc.gpsimd.tensor_copy(y7[:, :, 0:10, 3:4],
                                  qk9[:, :, 1:11, 1:2])
            nc.gpsimd.tensor_copy(y7[:, :, 10:11, 3:4],
                                  qk9[:, :, 10:11, 1:2])
            # ---- E_tap = E8 repeated over tau ----
            E_tap = p5.tile([128, W], F32, tag="etap")
            E7 = E_tap[:].rearrange("p (ch j f) -> p ch j f", j=NS, f=4)
            nc.gpsimd.tensor_copy(
                E7[:], E8[:].unsqueeze(-1).broadcast_to([128, CH, NS, 4]))

            # ---- feat = exp(min(y+b,0)) + relu(y+b) ----
            nc.vector.tensor_tensor(
                out=y_t[:].rearrange("p (c w) -> p c w", w=HL * NTB),
                in0=y_t[:].rearrange("p (c w) -> p c w", w=HL * NTB),
                in1=bias_t[:].unsqueeze(1).broadcast_to([128, NCH, HL * NTB]),
                op=ALU.add)
            m0 = p5.tile([128, W], F32, tag="m0")
            nc.vector.tensor_scalar(out=m0[:], in0=y_t[:], scalar1=0.0,
                                    scalar2=None, op0=ALU.min)
            nc.scalar.activation(m0[:], m0[:], AF.Exp)
            nc.vector.tensor_scalar(out=y_t[:], in0=y_t[:], scalar1=0.0,
                                    scalar2=None, op0=ALU.max)
            nc.vector.tensor_tensor(out=m0[:], in0=m0[:], in1=y_t[:], op=ALU.add)
            # A = feat * E * coef
            nc.vector.tensor_tensor(out=m0[:], in0=m0[:], in1=E_tap[:], op=ALU.mult)
            nc.vector.tensor_tensor(
                out=m0[:].rearrange("p (c w) -> p c w", w=HL * NTB),
                in0=m0[:].rearrange("p (c w) -> p c w", w=HL * NTB),
                in1=coef_t[:].unsqueeze(1).broadcast_to([128, NCH, HL * NTB]),
                op=ALU.mult)

            # ---- bypass: A_byp = SE * bp * feat0 (feat0 from qk_0) ----
            ab = p5.tile([128, CH], F32, tag="ab")
            t0 = p5.tile([128, CH], F32, tag="t0")
            nc.vector.tensor_scalar(out=ab[:], in0=qk8[:, :, 0:1].squeeze(-1), scalar1=0.0,
                                    scalar2=None, op0=ALU.min)
            nc.scalar.activation(ab[:], ab[:], AF.Exp)
            nc.vector.tensor_scalar(out=t0[:], in0=qk8[:, :, 0:1].squeeze(-1), scalar1=0.0,
                                    scalar2=None, op0=ALU.max)
            nc.vector.tensor_tensor(out=ab[:], in0=ab[:], in1=t0[:], op=ALU.add)
            nc.vector.tensor_tensor(out=ab[:], in0=ab[:], in1=SE_tm[:], op=ALU.mult)
            nc.vector.tensor_tensor(
                out=ab[:].rearrange("p (c h) -> p c h", h=HL),
                in0=ab[:].rearrange("p (c h) -> p c h", h=HL),
                in1=bp_t[:].unsqueeze(1).broadcast_to([128, NCH, HL]),
                op=ALU.mult)

            # ---- z = sum|A| + A_byp + SE*1e-6 ; recip ; normalize ----
            z_t = p5.tile([128, CH], F32, tag="z")
            nc.vector.tensor_reduce(
                z_t[:].rearrange("p (c h) -> p c h", h=HL).unsqueeze(-1),
                m0[:].rearrange("p (c h t) -> p c h t", h=HL, t=NTB),
                axis=AX.X, op=ALU.add, apply_absolute_value=True)
            nc.vector.tensor_tensor(out=z_t[:], in0=z_t[:], in1=ab[:], op=ALU.add)
            nc.vector.scalar_tensor_tensor(
                out=z_t[:], in0=SE_tm[:], scalar=1e-6, in1=z_t[:],
                op0=ALU.mult, op1=ALU.add)
            nc.vector.reciprocal(z_t[:], z_t[:])
            nc.vector.tensor_tensor(
                out=m0[:].rearrange("p (ch t) -> p ch t", t=NTB),
                in0=m0[:].rearrange("p (ch t) -> p ch t", t=NTB),
                in1=z_t[:].unsqueeze(-1).broadcast_to([128, CH, NTB]),
                op=ALU.mult)
            nc.vector.tensor_tensor(out=ab[:], in0=ab[:], in1=z_t[:], op=ALU.mult)

            # ---- group taps -> offsets: A_tm [128, (ch, omega)] ----
            A_tm = p5.tile([128, NCH * HL * NOFF], F32, tag="atm")
            A8 = A_tm[:].rearrange("p (ch o) -> p ch o", o=NOFF)
            A9 = A_tm[:].rearrange("p (ch e two) -> p ch e two", e=NS, two=2)
            m7 = m0[:].rearrange("p (ch j f) -> p ch j f", j=NS, f=4)
            # omega=0: sum over tau=0 column + bypass
            nc.vector.tensor_reduce(A8[:, :, 0:1].unsqueeze(-1),
                                    m7[:, :, :, 0:1].transpose([0, 1, 3, 2]),
                                    axis=AX.X, op=ALU.add)
            nc.vector.tensor_tensor(out=A8[:, :, 0:1].squeeze(-1),
                                    in0=A8[:, :, 0:1].squeeze(-1),
                                    in1=ab[:], op=ALU.add)
            # omega idx1 (off=1): tau=1 j=0
            nc.vector.tensor_copy(A8[:, :, 1:2], m7[:, :, 0:1, 1:2].squeeze(-1))
            # even omegas oi=2e (e=1..10): tau=1 (j=e>=1) + tau=2 (j=e-1<=9)
            nc.vector.tensor_tensor(out=A9[:, :, 1:11, 0:1],
                                    in0=m7[:, :, 1:11, 1:2],
                                    in1=m7[:, :, 0:10, 2:3], op=ALU.add)
            # odd omegas oi=2e+1 (e=1..10): tau=3 j=e-1
            nc.vector.tensor_copy(A9[:, :, 1:11, 1:2], m7[:, :, 0:10, 3:4])

            # ======== P6: transpose A_tm -> A_stage ========
            for c in range(NCH):
                for t in range(2):
                    ps2 = p6ps.tile([88, 128], F32, tag="tpb")
                    nc.tensor.transpose(
                        ps2[:],
                        A_tm[:, 176 * c + 88 * t:176 * c + 88 * (t + 1)],
                        ident_f[:])
                    nc.scalar.copy(A_stage[t][:, 128 * c:128 * (c + 1)], ps2[:])

        S_e.close()   # E/qk_tm freed

        if PHASE_LIMIT < 7:
            S_e.close() if False else None
            S_as.close()
            nc.compile()
            return nc
        # ======== P7: AV ========
        ofm = S.enter_context(tc.tile_pool(name="ofm", bufs=1))
        out_fm = [ofm.tile([128, N], F32, tag=f"ofm{p}", name=f"ofm{p}")
                  for p in range(NPAIR)]
        v2p = S.enter_context(tc.tile_pool(name="v2p", bufs=1))
        v2 = []
        for p in range(NPAIR):
            v2t = v2p.tile([128, N], F32R, tag=f"v2_{p}", name=f"v2_{p}")
            nc.sync.dma_start(v2t[:], v_d[p, :, :])
            v2.append(v2t)
        with tc.tile_pool(name="apk", bufs=1) as apk, \
             tc.tile_pool(name="p7", bufs=6) as p7, \
             tc.tile_pool(name="p7ps", bufs=4, space="PSUM") as p7ps, \
             tc.tile_pool(name="p7po", bufs=2, space="PSUM") as p7po:
            for p in range(NPAIR):
                # pack this pair's A rows 32-aligned: tile g holds offsets
                # 4g..4g+3 at partition bases 0/32/64/96 (2 rows each)
                packs = []
                for g in range((NOFF + 2) // 3):
                    pk = apk.tile([128, N], F32R, tag=f"apk{g}", name=f"apk{p}_{g}")
                    packs.append(pk)
                for oi in range(NOFF):
                    tl = A_stage[p // 2]
                    r0 = (2 * (p % 2)) * NOFF + oi
                    pk = packs[oi // 3]
                    rb = 32 * (oi % 3)
                    nc.sync.dma_start(pk[rb:rb + 1, :], tl[r0:r0 + 1, :])
                    nc.sync.dma_start(pk[rb + 1:rb + 2, :],
                                      tl[r0 + NOFF:r0 + NOFF + 1, :])
                for nb in range(NNB):
                    n0 = 512 * nb
                    po = p7po.tile([128, 512], F32, tag="avo")
                    valid = [(oi, off) for oi, off in enumerate(OFFSETS)
                             if off < n0 + 512]
                    for vi, (oi, off) in enumerate(valid):
                        pk = packs[oi // 3]
                        rb = 32 * (oi % 3)
                        pa = p7ps.tile([128, 512], F32, tag="aexp")
                        nc.tensor.matmul(
                            pa[:], opair[rb:rb + 2, :], pk[rb:rb + 2, n0:n0 + 512],
                            start=True, stop=True)
                        tmp = p7.tile([128, 512], F32R, tag="avt")
                        pre = max(0, off - n0)
                        if pre > 0:
                            nc.gpsimd.memset(tmp[:, 0:pre].bitcast(F32), 0.0)
                        nc.vector.tensor_tensor(
                            out=tmp[:, pre:512],
                            in0=v2[p][:, n0 + pre - off:n0 + 512 - off],
                            in1=pa[:, pre:512], op=ALU.mult)
                        nc.tensor.matmul(
                            po[:], ident[:], tmp[:],
                            start=(vi == 0), stop=(vi == len(valid) - 1))
                    nc.scalar.copy(out_fm[p][:, n0:n0 + 512], po[:])
        S_as.close()  # A_stage freed

        if PHASE_LIMIT < 8:
            nc.compile()
            return nc
        # ======== P8: gate + output projection ========
        with tc.tile_pool(name="p8", bufs=3) as p8, \
             tc.tile_pool(name="p8g", bufs=1) as p8g, \
             tc.tile_pool(name="p8ps", bufs=4, space="PSUM") as p8ps:
            gg = []
            for p in range(NPAIR):
                gt = p8.tile([128, N], F32, tag="gld")
                nc.sync.dma_start(gt[:], gate_d[p, :, :])
                ggt = p8g.tile([128, N], F32R, tag=f"gg{p}", name=f"gg{p}")
                nc.vector.tensor_tensor(out=ggt[:], in0=out_fm[p][:],
                                        in1=gt[:], op=ALU.mult)
                gg.append(ggt)
            wo_r = []
            for p in range(NPAIR):
                wor = p8g.tile([128, D], F32R, tag=f"wor{p}", name=f"wor{p}")
                nc.sync.dma_start(wor[:], woT[128 * p:128 * (p + 1), :])
                wo_r.append(wor)
            for dc in range(D // 128):
                for nb in range(NNB):
                    ps = p8ps.tile([128, 512], F32, tag="yps")
                    for p in range(NPAIR):
                        nc.tensor.matmul(
                            ps[:], wo_r[p][:, 128 * dc:128 * (dc + 1)],
                            gg[p][:, 512 * nb:512 * (nb + 1)],
                            start=(p == 0), stop=(p == NPAIR - 1))
                    yt = p8.tile([128, 512], F32, tag="yt")
                    nc.scalar.copy(yt[:], ps[:])
                    nc.sync.dma_start(
                        y_fm[128 * dc:128 * (dc + 1),
                             512 * nb:512 * (nb + 1)], yt[:])
    nc.compile()
    return nc


# ===========================================================================
# host side
# ===========================================================================

_OPAIR = np.zeros((128, 128), np.float32)
for _q in range(4):
    _OPAIR[32 * _q, 0:64] = 1.0
    _OPAIR[32 * _q + 1, 64:128] = 1.0


def _make_inputs(x, W_qkv, b_qkv, W_out, W_gate, b_gate, scale_gain, W_qscale,
                 identity_bypass, pos_bias, b, g):
    hg0 = g * HL
    rows = slice(hg0 * HD, (hg0 + HL) * HD)
    Wq = W_qkv[0 * D:1 * D][rows]
    Wk = W_qkv[1 * D:2 * D][rows]
    Wv = W_qkv[2 * D:3 * D][rows]
    Wg = W_gate[rows]
    wTv = np.concatenate([Wq, Wk, Wv, Wg], axis=0).T.copy()
    woTv = W_out[:, rows].T.copy()

    wqsv = np.zeros((128, 2 * NS), np.float32)
    wqsv[0:64, 0:NS] = W_qscale.T
    wqsv[64:128, NS:2 * NS] = W_qscale.T

    prior = np.zeros((HL, NS), np.float32)
    for h in range(HL):
        prior[h] = scale_gain[:, hg0 + h]
    prior_v = np.broadcast_to(prior.reshape(1, -1), (128, HL * NS)).copy()

    bias = np.zeros((HL, NTB), np.float32)
    coef = np.zeros((HL, NTB), np.float32)
    for h in range(HL):
        for (j, tau, off, full_idx) in TAPS:
            bias[h, full_idx] = pos_bias[full_idx, hg0 + h]
            coef[h, full_idx] = D4[tau]
    bias_v = np.broadcast_to(bias.reshape(1, -1), (128, HL * NTB)).copy()
    coef_v = np.broadcast_to(coef.reshape(1, -1), (128, HL * NTB)).copy()

    bqkv = np.zeros((128, 12), np.float32)
    for sect, bb in enumerate([b_qkv[0:D], b_qkv[D:2 * D], b_qkv[2 * D:3 * D]]):
        sl = bb[rows]
        for pair in range(NPAIR):
            bqkv[:, sect * 4 + pair] = sl[128 * pair:128 * (pair + 1)]
    bgate_v = np.zeros((128, NPAIR), np.float32)
    gsl = b_gate[rows]
    for pair in range(NPAIR):
        bgate_v[:, pair] = gsl[128 * pair:128 * (pair + 1)]

    bp = np.log1p(np.exp(identity_bypass[hg0:hg0 + HL])).astype(np.float32)
    bp_v = np.broadcast_to(bp.reshape(1, -1), (128, HL)).copy()

    return {
        "xT": np.ascontiguousarray(x[b].T),
        "wT": np.ascontiguousarray(wTv),
        "woT": np.ascontiguousarray(woTv),
        "wqs": wqsv,
        "prior_c": np.ascontiguousarray(prior_v),
        "bias_c": np.ascontiguousarray(bias_v),
        "coef_c": np.ascontiguousarray(coef_v),
        "bqkv_c": bqkv,
        "bgate_c": bgate_v,
        "bp_c": np.ascontiguousarray(bp_v),
        "opair_c": _OPAIR,
    }


def kernel(x, W_qkv, b_qkv, W_out, b_out, W_gate, b_gate, scale_gain, W_qscale,
           identity_bypass, pos_bias):
    x = np.asarray(x, np.float32)
    args = [np.asarray(a, np.float32) for a in
            (W_qkv, b_qkv, W_out, W_gate, b_gate, scale_gain, W_qscale,
             identity_bypass, pos_bias)]
    (W_qkv, b_qkv, W_out, W_gate, b_gate, scale_gain, W_qscale,
     identity_bypass, pos_bias) = args

    if "nc" not in _KERNEL_CACHE:
        _KERNEL_CACHE["nc"] = build_kernel()
    nc = _KERNEL_CACHE["nc"]

    in_maps = []
    for core in range(8):
        b, g = core % 4, core // 4
        in_maps.append(_make_inputs(x, W_qkv, b_qkv, W_out, W_gate, b_gate,
                                    scale_gain, W_qscale, identity_bypass,
                                    pos_bias, b, g))
    res = run_bass_kernel_spmd(nc, in_maps, list(range(8)))

    out = np.zeros((B, N, D), np.float32)
    for core in range(8):
        b = core % 4
        out[b] += res.results[core]["y_fm"].T
    out += np.asarray(b_out, np.float32)
    return out



# revision 13
# speedup vs baseline: 2.0307x; 2.0307x over previous
"""DWARF attention Trainium2 Bass kernel (v2, bf16).

Sharding: 8 cores = 4 batches x 2 head-halves (8 local heads each).
Per-core dataflow (feature-major = [feature rows, token cols]):
  P1 proj:  q/k/v/gate = W^T.T @ xT on PE (bf16), ACT evictions w/ bias+sigmoid
            k/v evicted into left-zero-padded resident tiles (shifted reads)
  P2 E:     E = exp(q_offset + prior) token-major (bf16), SE row-sums (f32)
  P3 qk:    per offset: 4 pair-products (DVE bf16) -> 4 pair-sum matmuls into
            one psum tile at row bases {0,32,64,96} -> 1 ACT evict [98,512]
            -> 2 batched strided-partition DMAs into qk_T [88,N] f32
  P4 tm:    PE-transpose qk_T -> qk_tm token-major bf16
  P5 feat:  tap-gathers, feat=elu(qk+b)+1, A=E*coef*feat/z (DVE/ACT bf16)
  P6 A_T:   PE-transpose A_tm (f32) -> A_stage [88,N] bf16
  P7 AV:    per (pair,nb,off): sel-matmul expand (A_stage -> 128 rows psum),
            DVE mul with padded v2, PE identity accumulate
  P8 out:   gg = out_fm*gate; y_fm = Wout^T.T @ gg (PE bf16) -> DRAM f32
Host: shard, pre-transpose weights to bf16, build sel/tap tables, reduce
head-halves.
"""
from contextlib import ExitStack

import ml_dtypes
import numpy as np

import concourse.bass as bass
import concourse.mybir as mybir
import concourse.tile as tile
from concourse import bacc
from concourse.bass_utils import run_bass_kernel_spmd
from concourse.masks import make_identity

F32 = mybir.dt.float32
BF16 = mybir.dt.bfloat16
AF = mybir.ActivationFunctionType
ALU = mybir.AluOpType
AX = mybir.AxisListType

B, N, D, H = 4, 2048, 1024, 16
HD = 64
NS = 11
HL = 8
NPAIR = 4
PAD = 1536
NPADCOLS = PAD + N
D4 = [0.4829629131445341, 0.8365163037378079, 0.2241438680420134, -0.1294095225512604]

TAPS = []
for _j in range(NS):
    _d = 1 << _j
    for _tau in range(4):
        _off = _d * _tau
        if _off != 0 and _off >= N:
            continue
        TAPS.append((_j, _tau, _off, 4 * _j + _tau))
NTAP = len(TAPS)            # 42
NTB = 44                    # full (j, tau) grid; invalid slots get coef 0
OFFSETS = sorted({t[2] for t in TAPS})
NOFF = len(OFFSETS)         # 22
OFF_IDX = {o: i for i, o in enumerate(OFFSETS)}
NCH = N // 128
NNB = N // 512

_KERNEL_CACHE = {}


def build_kernel(dbg=False):
    nc = bacc.Bacc("TRN2", target_bir_lowering=False, debug=False, num_devices=8)

    xT = nc.dram_tensor("xT", [D, N], BF16, kind="ExternalInput")
    wT = nc.dram_tensor("wT", [D, 4 * HL * HD], BF16, kind="ExternalInput")
    woT = nc.dram_tensor("woT", [HL * HD, D], BF16, kind="ExternalInput")
    wqs = nc.dram_tensor("wqs", [128, 2 * NS], BF16, kind="ExternalInput")
    sel_c = nc.dram_tensor("sel_c", [128, 2 * NOFF * 128], BF16,
                           kind="ExternalInput")
    prior_c = nc.dram_tensor("prior_c", [128, HL * NS], F32, kind="ExternalInput")
    bias_c = nc.dram_tensor("bias_c", [128, HL * NTB], F32, kind="ExternalInput")
    coef_c = nc.dram_tensor("coef_c", [128, HL * NTB], F32, kind="ExternalInput")
    bqkv_c = nc.dram_tensor("bqkv_c", [128, 12], F32, kind="ExternalInput")
    bgate_c = nc.dram_tensor("bgate_c", [128, NPAIR], F32, kind="ExternalInput")
    bp_c = nc.dram_tensor("bp_c", [128, HL], F32, kind="ExternalInput")

    y_fm = nc.dram_tensor("y_fm", [D, N], F32, kind="ExternalOutput")
    if dbg:
        d_q2 = nc.dram_tensor("d_q2", [128, N], BF16, kind="ExternalOutput")
        d_k2 = nc.dram_tensor("d_k2", [128, NPADCOLS], BF16,
                              kind="ExternalOutput")
        d_v2 = nc.dram_tensor("d_v2", [128, NPADCOLS], BF16,
                              kind="ExternalOutput")
        d_gate = nc.dram_tensor("d_gate", [128, N], BF16, kind="ExternalOutput")
        d_qkT = nc.dram_tensor("d_qkT", [88, N], F32, kind="ExternalOutput")
        d_E = nc.dram_tensor("d_E", [128, NCH * HL * NS], BF16,
                             kind="ExternalOutput")
        d_qktm = nc.dram_tensor("d_qktm", [128, NCH * 2 * 88], BF16,
                                kind="ExternalOutput")
        d_ast = nc.dram_tensor("d_ast", [88, N], BF16, kind="ExternalOutput")
        d_ofm = nc.dram_tensor("d_ofm", [128, N], BF16, kind="ExternalOutput")

    with tile.TileContext(nc) as tc, ExitStack() as S:
        # ---- persistent constants (small) ----
        const = S.enter_context(tc.tile_pool(name="const", bufs=1))
        ident_f = const.tile([128, 128], F32)
        make_identity(nc, ident_f)
        ident_b = const.tile([128, 128], BF16)
        nc.vector.tensor_copy(ident_b[:], ident_f[:])
        ones_f = const.tile([128, 2], F32)
        nc.vector.memset(ones_f[:], 0.0)
        nc.vector.memset(ones_f[0:64, 0:1], 1.0)
        nc.vector.memset(ones_f[64:128, 1:2], 1.0)
        ones2 = const.tile([128, 2], BF16)
        nc.vector.tensor_copy(ones2[:], ones_f[:])
        sel_t = const.tile([128, 2 * NOFF * 128], BF16)
        nc.sync.dma_start(sel_t[:], sel_c[:])
        wqs_r = const.tile([128, 2 * NS], BF16)
        nc.sync.dma_start(wqs_r[:], wqs[:])
        prior_t = const.tile([128, HL * NS], F32)
        nc.sync.dma_start(prior_t[:], prior_c[:])
        bias_t = const.tile([128, HL * NTB], F32)
        nc.sync.dma_start(bias_t[:], bias_c[:])
        coef_t = const.tile([128, HL * NTB], F32)
        nc.sync.dma_start(coef_t[:], coef_c[:])
        bqkv_t = const.tile([128, 12], F32)
        nc.sync.dma_start(bqkv_t[:], bqkv_c[:])
        bgate_t = const.tile([128, NPAIR], F32)
        nc.sync.dma_start(bgate_t[:], bgate_c[:])
        bp_t = const.tile([128, HL], F32)
        nc.sync.dma_start(bp_t[:], bp_c[:])

        # ---- persistent activations ----
        big = S.enter_context(tc.tile_pool(name="big", bufs=1, side="right"))
        k2 = [big.tile([128, NPADCOLS], BF16, tag=f"k2_{p}", name=f"k2_{p}")
              for p in range(NPAIR)]
        v2 = [big.tile([128, NPADCOLS], BF16, tag=f"v2_{p}", name=f"v2_{p}")
              for p in range(NPAIR)]
        gate = [big.tile([128, N], BF16, tag=f"g_{p}", name=f"g_{p}")
                for p in range(NPAIR)]
        out_fm = [big.tile([128, N], BF16, tag=f"o_{p}", name=f"o_{p}")
                  for p in range(NPAIR)]
        for p in range(NPAIR):
            nc.vector.memset(k2[p][:, 0:PAD], 0.0)
            nc.vector.memset(v2[p][:, 0:PAD], 0.0)

        S_as = ExitStack()
        arow = S_as.enter_context(tc.tile_pool(name="arow", bufs=1, side="right"))
        A_stage = [arow.tile([88, N], BF16, tag=f"ast{t}", name=f"ast{t}")
                   for t in range(2)]

        S_e = ExitStack()
        ep = S_e.enter_context(tc.tile_pool(name="ep", bufs=1))
        E_tm = ep.tile([128, NCH * HL * NS], BF16)
        SE_tm = ep.tile([128, NCH * HL], F32)
        qk_Tp = S_e.enter_context(tc.tile_pool(name="qkTp", bufs=1, side="right"))
        qk_T = [qk_Tp.tile([88, N], F32, tag=f"qkT{t}", name=f"qkT{t}")
                for t in range(2)]

        # ======== P1: projections ========
        S_q = ExitStack()
        qp = S_q.enter_context(tc.tile_pool(name="qp", bufs=1, side="right"))
        q2 = [qp.tile([128, N], BF16, tag=f"q2_{p}", name=f"q2_{p}")
              for p in range(NPAIR)]
        with tc.tile_pool(name="p1x", bufs=1) as p1x, \
             tc.tile_pool(name="p1", bufs=2) as p1, \
             tc.tile_pool(name="p1ps", bufs=4, space="PSUM") as p1ps:
            xr = []
            for kc in range(D // 128):
                xrt = p1x.tile([128, N], BF16, tag=f"xr{kc}", name=f"xr{kc}")
                nc.sync.dma_start(xrt[:], xT[128 * kc:128 * (kc + 1), :])
                xr.append(xrt)
            for fc in range(16):
                wr = p1.tile([128, 8 * 128], BF16, tag="wr")
                nc.sync.dma_start(
                    wr[:].rearrange("p (a m) -> p a m", a=8),
                    wT[:, 128 * fc:128 * (fc + 1)]
                    .rearrange("(a p) m -> p a m", p=128))
                sect, pair = fc // 4, fc % 4
                for nb in range(NNB):
                    ps = p1ps.tile([128, 512], F32, tag="proj")
                    for kc in range(D // 128):
                        nc.tensor.matmul(
                            ps[:], wr[:, 128 * kc:128 * (kc + 1)],
                            xr[kc][:, 512 * nb:512 * (nb + 1)],
                            start=(kc == 0), stop=(kc == 7))
                    sl = slice(512 * nb, 512 * (nb + 1))
                    slp = slice(PAD + 512 * nb, PAD + 512 * (nb + 1))
                    if sect == 0:
                        nc.scalar.activation(q2[pair][:, sl], ps[:],
                                             AF.Identity, bias=bqkv_t[:, fc:fc + 1])
                    elif sect == 1:
                        nc.scalar.activation(k2[pair][:, slp], ps[:],
                                             AF.Identity, bias=bqkv_t[:, fc:fc + 1])
                    elif sect == 2:
                        nc.scalar.activation(v2[pair][:, slp], ps[:],
                                             AF.Identity, bias=bqkv_t[:, fc:fc + 1])
                    else:
                        nc.scalar.activation(gate[pair][:, sl], ps[:], AF.Sigmoid,
                                             bias=bgate_t[:, pair:pair + 1])

        if dbg:
            nc.sync.dma_start(d_q2[:], q2[0][:])
            nc.sync.dma_start(d_k2[:], k2[0][:])
            nc.sync.dma_start(d_v2[:], v2[0][:])
            nc.sync.dma_start(d_gate[:], gate[0][:])

        # ======== P2: E = exp(q_offset + prior), SE ========
        with tc.tile_pool(name="p2", bufs=2) as p2, \
             tc.tile_pool(name="p2ps", bufs=2, space="PSUM") as p2ps:
            for c in range(NCH):
                ps = p2ps.tile([128, HL * NS], F32, tag="qs")
                for p in range(NPAIR):
                    nc.tensor.matmul(
                        ps[:, 2 * NS * p:2 * NS * (p + 1)],
                        q2[p][:, 128 * c:128 * (c + 1)], wqs_r[:],
                        start=True, stop=True)
                et = p2.tile([128, HL * NS], F32, tag="et")
                nc.vector.tensor_tensor(out=et[:], in0=ps[:], in1=prior_t[:],
                                        op=ALU.add)
                nc.scalar.activation(E_tm[:, HL * NS * c:HL * NS * (c + 1)],
                                     et[:], AF.Exp)
            nc.vector.tensor_reduce(
                SE_tm[:].rearrange("p (c h) -> p c h", h=HL).unsqueeze(-1),
                E_tm[:].rearrange("p (c h s) -> p c h s", h=HL, s=NS),
                axis=AX.X, op=ALU.add)

        # ======== P3: qk scores -> qk_T assembly ========
        with tc.tile_pool(name="p3", bufs=2) as p3, \
             tc.tile_pool(name="p3s", bufs=2) as p3s, \
             tc.tile_pool(name="p3ps", bufs=4, space="PSUM") as p3ps:
            for oi, off in enumerate(OFFSETS):
                prods = []
                for p in range(NPAIR):
                    prod = p3.tile([128, N], BF16, tag=f"prod{p}")
                    nc.vector.tensor_tensor(
                        out=prod[:], in0=q2[p][:],
                        in1=k2[p][:, PAD - off:PAD - off + N], op=ALU.mult)
                    prods.append(prod)
                st = p3s.tile([128, N], F32, tag="stage")
                for nb in range(NNB):
                    ps = p3ps.tile([128, 512], F32, tag="qkps")
                    for p in range(NPAIR):
                        nc.tensor.matmul(
                            ps[32 * p:32 * p + 2, :], ones2[:],
                            prods[p][:, 512 * nb:512 * (nb + 1)],
                            start=True, stop=True,
                            tile_position=(0, 32 * p))
                    nc.scalar.copy(st[:, 512 * nb:512 * (nb + 1)], ps[:])
                for t in range(2):
                    for ph in range(2):
                        p = 2 * t + ph
                        dst = qk_T[t][:].rearrange(
                            "(a j o) n -> a j o n", a=2, j=2)[ph, :, oi, :]
                        nc.sync.dma_start(dst, st[32 * p:32 * p + 2, :])
        S_q.close()  # q2 released
        if dbg:
            nc.sync.dma_start(d_qkT[:], qk_T[0][:])
            nc.sync.dma_start(d_E[:], E_tm[:])

        # ======== P4: transpose qk_T -> token-major ========
        qk_tm = ep.tile([128, NCH * 2 * 88], BF16, name="qk_tm")
        with tc.tile_pool(name="p4ps", bufs=4, space="PSUM") as p4ps:
            for c in range(NCH):
                for t in range(2):
                    ps = p4ps.tile([128, 88], F32, tag="tp")
                    nc.tensor.transpose(
                        ps[:], qk_T[t][:, 128 * c:128 * (c + 1)],
                        ident_f[0:88, 0:88])
                    nc.scalar.copy(
                        qk_tm[:, 176 * c + 88 * t:176 * c + 88 * (t + 1)],
                        ps[:])

        # ======== P5: feat chain (token-major) ========
        with tc.tile_pool(name="p5", bufs=1) as p5, \
             tc.tile_pool(name="p6ps", bufs=4, space="PSUM") as p6ps:
            W = NCH * HL * NTB          # 5632
            CH = NCH * HL               # 128 (c,h) groups
            qk8 = qk_tm[:].rearrange("p (ch o) -> p ch o", o=NOFF)
            qk9 = qk_tm[:].rearrange("p (ch e two) -> p ch e two", e=NS, two=2)
            E8 = E_tm[:].rearrange("p (ch s) -> p ch s", s=NS)

            # ---- expand taps: y_t [128, (ch, j, tau)] ----
            y_t = p5.tile([128, W], BF16, tag="y")
            y7 = y_t[:].rearrange("p (ch j f) -> p ch j f", j=NS, f=4)
            # tau=0: qk_0 for all j
            nc.gpsimd.tensor_copy(
                y7[:, :, :, 0:1],
                qk8[:, :, 0:1].unsqueeze(2).broadcast_to([128, CH, NS, 1]))
            # tau=1: j=0 -> oi1 ; j=1 -> oi2 ; j>=2 -> oi=2j
            nc.gpsimd.tensor_copy(y7[:, :, 0:1, 1:2],
                                  qk8[:, :, 1:2].unsqueeze(2))
            nc.gpsimd.tensor_copy(y7[:, :, 1:2, 1:2],
                                  qk8[:, :, 2:3].unsqueeze(2))
            nc.gpsimd.tensor_copy(y7[:, :, 2:11, 1:2],
                                  qk9[:, :, 2:11, 0:1])
            # tau=2: j=0 -> oi2 ; j=1..9 -> oi=2j+2 ; j=10 invalid (dup j=9)
            nc.gpsimd.tensor_copy(y7[:, :, 0:1, 2:3],
                                  qk8[:, :, 2:3].unsqueeze(2))
            nc.gpsimd.tensor_copy(y7[:, :, 1:10, 2:3],
                                  qk9[:, :, 2:11, 0:1])
            nc.gpsimd.tensor_copy(y7[:, :, 10:11, 2:3],
                                  qk9[:, :, 10:11, 0:1])
            # tau=3: j=0..9 -> two=1, e=j+1 ; j=10 invalid (dup)
            nc.gpsimd.tensor_copy(y7[:, :, 0:10, 3:4],
                                  qk9[:, :, 1:11, 1:2])
            nc.gpsimd.tensor_copy(y7[:, :, 10:11, 3:4],
                                  qk9[:, :, 10:11, 1:2])

            # ---- feat = exp(min(y+b,0)) + relu(y+b) ----
            nc.vector.tensor_tensor(
                out=y_t[:].rearrange("p (c w) -> p c w", w=HL * NTB),
                in0=y_t[:].rearrange("p (c w) -> p c w", w=HL * NTB),
                in1=bias_t[:].unsqueeze(1).broadcast_to([128, NCH, HL * NTB]),
                op=ALU.add)
            m0 = p5.tile([128, W], BF16, tag="m0")
            nc.vector.tensor_scalar(out=m0[:], in0=y_t[:], scalar1=0.0,
                                    scalar2=None, op0=ALU.min)
            nc.scalar.activation(m0[:], m0[:], AF.Exp)
            nc.vector.tensor_scalar(out=y_t[:], in0=y_t[:], scalar1=0.0,
                                    scalar2=None, op0=ALU.max)
            nc.vector.tensor_tensor(out=m0[:], in0=m0[:], in1=y_t[:], op=ALU.add)
            # A = feat * E * coef  (E broadcast over tau, coef over chunks)
            nc.vector.tensor_tensor(
                out=m0[:].rearrange("p (ch s f) -> p ch s f", s=NS, f=4),
                in0=m0[:].rearrange("p (ch s f) -> p ch s f", s=NS, f=4),
                in1=E8[:].unsqueeze(-1).broadcast_to([128, CH, NS, 4]),
                op=ALU.mult)
            nc.vector.tensor_tensor(
                out=m0[:].rearrange("p (c w) -> p c w", w=HL * NTB),
                in0=m0[:].rearrange("p (c w) -> p c w", w=HL * NTB),
                in1=coef_t[:].unsqueeze(1).broadcast_to([128, NCH, HL * NTB]),
                op=ALU.mult)

            # ---- bypass: A_byp = SE * bp * feat0 (feat0 from qk_0) ----
            ab = p5.tile([128, CH], F32, tag="ab")
            t0 = p5.tile([128, CH], F32, tag="t0")
            nc.vector.tensor_scalar(out=ab[:], in0=qk8[:, :, 0:1].squeeze(-1),
                                    scalar1=0.0, scalar2=None, op0=ALU.min)
            nc.scalar.activation(ab[:], ab[:], AF.Exp)
            nc.vector.tensor_scalar(out=t0[:], in0=qk8[:, :, 0:1].squeeze(-1),
                                    scalar1=0.0, scalar2=None, op0=ALU.max)
            nc.vector.tensor_tensor(out=ab[:], in0=ab[:], in1=t0[:], op=ALU.add)
            nc.vector.tensor_tensor(out=ab[:], in0=ab[:], in1=SE_tm[:], op=ALU.mult)
            nc.vector.tensor_tensor(
                out=ab[:].rearrange("p (c h) -> p c h", h=HL),
                in0=ab[:].rearrange("p (c h) -> p c h", h=HL),
                in1=bp_t[:].unsqueeze(1).broadcast_to([128, NCH, HL]),
                op=ALU.mult)

            # ---- z = sum|A| + A_byp + SE*1e-6 ; recip ; normalize ----
            z_t = p5.tile([128, CH], F32, tag="z")
            nc.vector.tensor_reduce(
                z_t[:].rearrange("p (c h) -> p c h", h=HL).unsqueeze(-1),
                m0[:].rearrange("p (c h t) -> p c h t", h=HL, t=NTB),
                axis=AX.X, op=ALU.add, apply_absolute_value=True)
            nc.vector.tensor_tensor(out=z_t[:], in0=z_t[:], in1=ab[:], op=ALU.add)
            nc.vector.scalar_tensor_tensor(
                out=z_t[:], in0=SE_tm[:], scalar=1e-6, in1=z_t[:],
                op0=ALU.mult, op1=ALU.add)
            nc.vector.reciprocal(z_t[:], z_t[:])
            nc.vector.tensor_tensor(
                out=m0[:].rearrange("p (ch t) -> p ch t", t=NTB),
                in0=m0[:].rearrange("p (ch t) -> p ch t", t=NTB),
                in1=z_t[:].unsqueeze(-1).broadcast_to([128, CH, NTB]),
                op=ALU.mult)
            nc.vector.tensor_tensor(out=ab[:], in0=ab[:], in1=z_t[:], op=ALU.mult)

            # ---- group taps -> offsets: A_tm [128, (ch, omega)] f32 ----
            A_tm = p5.tile([128, NCH * HL * NOFF], F32, tag="atm")
            A8 = A_tm[:].rearrange("p (ch o) -> p ch o", o=NOFF)
            A9 = A_tm[:].rearrange("p (ch e two) -> p ch e two", e=NS, two=2)
            m7 = m0[:].rearrange("p (ch j f) -> p ch j f", j=NS, f=4)
            # omega=0: sum over tau=0 column + bypass
            nc.vector.tensor_reduce(A8[:, :, 0:1].unsqueeze(-1),
                                    m7[:, :, :, 0:1].transpose([0, 1, 3, 2]),
                                    axis=AX.X, op=ALU.add)
            nc.vector.tensor_tensor(out=A8[:, :, 0:1].squeeze(-1),
                                    in0=A8[:, :, 0:1].squeeze(-1),
                                    in1=ab[:], op=ALU.add)
            # omega idx1 (off=1): tau=1 j=0
            nc.vector.tensor_copy(A8[:, :, 1:2], m7[:, :, 0:1, 1:2].squeeze(-1))
            # even omegas oi=2e (e=1..10): tau=1 (j=e>=1) + tau=2 (j=e-1<=9)
            nc.vector.tensor_tensor(out=A9[:, :, 1:11, 0:1],
                                    in0=m7[:, :, 1:11, 1:2],
                                    in1=m7[:, :, 0:10, 2:3], op=ALU.add)
            # odd omegas oi=2e+1 (e=1..10): tau=3 j=e-1
            nc.vector.tensor_copy(A9[:, :, 1:11, 1:2], m7[:, :, 0:10, 3:4])

            # ======== P6: transpose A_tm -> A_stage ========
            for c in range(NCH):
                for t in range(2):
                    ps2 = p6ps.tile([88, 128], F32, tag="tpb")
                    nc.tensor.transpose(
                        ps2[:],
                        A_tm[:, 176 * c + 88 * t:176 * c + 88 * (t + 1)],
                        ident_f[:])
                    nc.scalar.copy(A_stage[t][:, 128 * c:128 * (c + 1)], ps2[:])

        if dbg:
            nc.sync.dma_start(d_qktm[:], qk_tm[:])
            nc.sync.dma_start(d_ast[:], A_stage[0][:])
        S_e.close()   # E/qk_tm/qk_T freed

        # ======== P7: AV ========
        sel_v = sel_t[:].rearrange("p (i m) -> p i m", i=2 * NOFF)
        with tc.tile_pool(name="p7", bufs=6) as p7, \
             tc.tile_pool(name="p7ps", bufs=4, space="PSUM") as p7ps, \
             tc.tile_pool(name="p7po", bufs=2, space="PSUM") as p7po:
            for p in range(NPAIR):
                t, ph = p // 2, p % 2
                for nb in range(NNB):
                    n0 = 512 * nb
                    po = p7po.tile([128, 512], F32, tag="avo")
                    valid = [(oi, off) for oi, off in enumerate(OFFSETS)
                             if off < n0 + 512]
                    for vi, (oi, off) in enumerate(valid):
                        pa = p7ps.tile([128, 512], F32, tag="aexp")
                        nc.tensor.matmul(
                            pa[:], sel_v[0:88, NOFF * ph + oi, :],
                            A_stage[t][:, n0:n0 + 512],
                            start=True, stop=True)
                        tmp = p7.tile([128, 512], BF16, tag="avt")
                        nc.vector.tensor_tensor(
                            out=tmp[:],
                            in0=v2[p][:, PAD + n0 - off:PAD + n0 + 512 - off],
                            in1=pa[:], op=ALU.mult)
                        nc.tensor.matmul(
                            po[:], ident_b[:], tmp[:],
                            start=(vi == 0), stop=(vi == len(valid) - 1))
                    nc.scalar.copy(out_fm[p][:, n0:n0 + 512], po[:])
        if dbg:
            nc.sync.dma_start(d_ofm[:], out_fm[0][:])
        S_as.close()  # A_stage freed

        # ======== P8: gate + output projection ========
        with tc.tile_pool(name="p8", bufs=3) as p8, \
             tc.tile_pool(name="p8g", bufs=1) as p8g, \
             tc.tile_pool(name="p8ps", bufs=4, space="PSUM") as p8ps:
            for p in range(NPAIR):
                nc.vector.tensor_tensor(out=out_fm[p][:], in0=out_fm[p][:],
                                        in1=gate[p][:], op=ALU.mult)
            wo_r = []
            for p in range(NPAIR):
                wor = p8g.tile([128, D], BF16, tag=f"wor{p}", name=f"wor{p}")
                nc.sync.dma_start(wor[:], woT[128 * p:128 * (p + 1), :])
                wo_r.append(wor)
            for dc in range(D // 128):
                for nb in range(NNB):
                    ps = p8ps.tile([128, 512], F32, tag="yps")
                    for p in range(NPAIR):
                        nc.tensor.matmul(
                            ps[:], wo_r[p][:, 128 * dc:128 * (dc + 1)],
                            out_fm[p][:, 512 * nb:512 * (nb + 1)],
                            start=(p == 0), stop=(p == NPAIR - 1))
                    yt = p8.tile([128, 512], F32, tag="yt")
                    nc.scalar.copy(yt[:], ps[:])
                    nc.sync.dma_start(
                        y_fm[128 * dc:128 * (dc + 1),
                             512 * nb:512 * (nb + 1)], yt[:])
    nc.compile()
    return nc


# ===========================================================================
# host side
# ===========================================================================

_SEL = np.zeros((128, 2 * NOFF * 128), np.float32)
for _ph in range(2):
    for _oi in range(NOFF):
        _i = NOFF * _ph + _oi
        _SEL[44 * _ph + _oi, 128 * _i:128 * _i + 64] = 1.0
        _SEL[44 * _ph + NOFF + _oi, 128 * _i + 64:128 * (_i + 1)] = 1.0


def _bf16(a):
    return np.ascontiguousarray(a).astype(ml_dtypes.bfloat16)


def _make_inputs(x, W_qkv, b_qkv, W_out, W_gate, b_gate, scale_gain, W_qscale,
                 identity_bypass, pos_bias, b, g):
    hg0 = g * HL
    rows = slice(hg0 * HD, (hg0 + HL) * HD)
    Wq = W_qkv[0 * D:1 * D][rows]
    Wk = W_qkv[1 * D:2 * D][rows]
    Wv = W_qkv[2 * D:3 * D][rows]
    Wg = W_gate[rows]
    wTv = np.concatenate([Wq, Wk, Wv, Wg], axis=0).T.copy()
    woTv = W_out[:, rows].T.copy()

    wqsv = np.zeros((128, 2 * NS), np.float32)
    wqsv[0:64, 0:NS] = W_qscale.T
    wqsv[64:128, NS:2 * NS] = W_qscale.T

    prior = np.zeros((HL, NS), np.float32)
    for h in range(HL):
        prior[h] = scale_gain[:, hg0 + h]
    prior_v = np.broadcast_to(prior.reshape(1, -1), (128, HL * NS)).copy()

    bias = np.zeros((HL, NTB), np.float32)
    coef = np.zeros((HL, NTB), np.float32)
    for h in range(HL):
        for (j, tau, off, full_idx) in TAPS:
            bias[h, full_idx] = pos_bias[full_idx, hg0 + h]
            coef[h, full_idx] = D4[tau]
    bias_v = np.broadcast_to(bias.reshape(1, -1), (128, HL * NTB)).copy()
    coef_v = np.broadcast_to(coef.reshape(1, -1), (128, HL * NTB)).copy()

    bqkv = np.zeros((128, 12), np.float32)
    for sect, bb in enumerate([b_qkv[0:D], b_qkv[D:2 * D], b_qkv[2 * D:3 * D]]):
        sl = bb[rows]
        for pair in range(NPAIR):
            bqkv[:, sect * 4 + pair] = sl[128 * pair:128 * (pair + 1)]
    bgate_v = np.zeros((128, NPAIR), np.float32)
    gsl = b_gate[rows]
    for pair in range(NPAIR):
        bgate_v[:, pair] = gsl[128 * pair:128 * (pair + 1)]

    bp = np.log1p(np.exp(identity_bypass[hg0:hg0 + HL])).astype(np.float32)
    bp_v = np.broadcast_to(bp.reshape(1, -1), (128, HL)).copy()

    return {
        "xT": _bf16(x[b].T),
        "wT": _bf16(wTv),
        "woT": _bf16(woTv),
        "wqs": _bf16(wqsv),
        "sel_c": _bf16(_SEL),
        "prior_c": np.ascontiguousarray(prior_v),
        "bias_c": np.ascontiguousarray(bias_v),
        "coef_c": np.ascontiguousarray(coef_v),
        "bqkv_c": bqkv,
        "bgate_c": bgate_v,
        "bp_c": np.ascontiguousarray(bp_v),
    }


def kernel(x, W_qkv, b_qkv, W_out, b_out, W_gate, b_gate, scale_gain, W_qscale,
           identity_bypass, pos_bias):
    x = np.asarray(x, np.float32)
    args = [np.asarray(a, np.float32) for a in
            (W_qkv, b_qkv, W_out, W_gate, b_gate, scale_gain, W_qscale,
             identity_bypass, pos_bias)]
    (W_qkv, b_qkv, W_out, W_gate, b_gate, scale_gain, W_qscale,
     identity_bypass, pos_bias) = args

    if "nc" not in _KERNEL_CACHE:
        _KERNEL_CACHE["nc"] = build_kernel()
    nc = _KERNEL_CACHE["nc"]

    in_maps = []
    for core in range(8):
        b, g = core % 4, core // 4
        in_maps.append(_make_inputs(x, W_qkv, b_qkv, W_out, W_gate, b_gate,
                                    scale_gain, W_qscale, identity_bypass,
                                    pos_bias, b, g))
    res = run_bass_kernel_spmd(nc, in_maps, list(range(8)))

    out = np.zeros((B, N, D), np.float32)
    for core in range(8):
        b = core % 4
        out[b] += res.results[core]["y_fm"].T
    out += np.asarray(b_out, np.float32)
    return out


# revision 17
# speedup vs baseline: 2.0524x; 1.0107x over previous
"""DWARF attention Trainium2 Bass kernel (v3, bf16 + pipelined halves).

Sharding: 8 cores = 4 batches x 2 head-halves (8 local heads each).
Per-core dataflow (feature-major = [feature rows, token cols]):
  P1 proj:  q/k/v/gate = W^T.T @ xT on PE (bf16), ACT evictions w/ bias+sigmoid
            k/v evicted into left-zero-padded resident tiles (shifted reads)
  P2 E:     E = exp(q_offset + prior) token-major (bf16), SE row-sums (f32),
            EC = E*coef tap table built off critical path
  P3 qk:    per offset: 4 pair-products (DVE bf16) -> 16 pair-sum matmuls into
            one 4-bank psum tile at row bases {0,32,64,96} -> 4 direct
            psum->SBUF row DMAs into qk_T [88,N] f32
  P4 tm:    PE-transpose qk_T -> qk_tm token-major bf16
  P5 feat:  (per token-half) tap-gathers, feat=elu(qk+b)+1, A=EC*feat/z
  P6 A_T:   (per half) PE-transpose A_tm (f32) -> A_stage [88,N] bf16
  P7 AV:    (per nb) per (pair,off): sel-matmul expand, DVE mul w/ padded v2,
            PE identity accumulate
  P8 out:   (per nb) gg = out_fm*gate; y_fm = Wout^T.T @ gg (PE bf16) -> DRAM
Host: shard, pre-transpose weights to bf16, build sel/tap tables, reduce
head-halves.
"""
from contextlib import ExitStack

import ml_dtypes
import numpy as np

import concourse.bass as bass
import concourse.mybir as mybir
import concourse.tile as tile
from concourse import bacc
from concourse.bass_utils import run_bass_kernel_spmd
from concourse.masks import make_identity

F32 = mybir.dt.float32
BF16 = mybir.dt.bfloat16
AF = mybir.ActivationFunctionType
ALU = mybir.AluOpType
AX = mybir.AxisListType

B, N, D, H = 4, 2048, 1024, 16
HD = 64
NS = 11
HL = 8
NPAIR = 4
PAD = 1536
NPADCOLS = PAD + N
D4 = [0.4829629131445341, 0.8365163037378079, 0.2241438680420134, -0.1294095225512604]

TAPS = []
for _j in range(NS):
    _d = 1 << _j
    for _tau in range(4):
        _off = _d * _tau
        if _off != 0 and _off >= N:
            continue
        TAPS.append((_j, _tau, _off, 4 * _j + _tau))
NTAP = len(TAPS)            # 42
NTB = 44                    # full (j, tau) grid; invalid slots get coef 0
OFFSETS = sorted({t[2] for t in TAPS})
NOFF = len(OFFSETS)         # 22
OFF_IDX = {o: i for i, o in enumerate(OFFSETS)}
NCH = N // 128
NNB = N // 512

_KERNEL_CACHE = {}


def build_kernel(dbg=False):
    nc = bacc.Bacc("TRN2", target_bir_lowering=False, debug=False, num_devices=8)

    xT = nc.dram_tensor("xT", [D, N], BF16, kind="ExternalInput")
    wT = nc.dram_tensor("wT", [D, 4 * HL * HD], BF16, kind="ExternalInput")
    woT = nc.dram_tensor("woT", [HL * HD, D], BF16, kind="ExternalInput")
    wqs = nc.dram_tensor("wqs", [128, 2 * NS], BF16, kind="ExternalInput")
    sel_c = nc.dram_tensor("sel_c", [128, 2 * NOFF * 128], BF16,
                           kind="ExternalInput")
    prior_c = nc.dram_tensor("prior_c", [128, HL * NS], F32, kind="ExternalInput")
    bias_c = nc.dram_tensor("bias_c", [128, HL * NTB], F32, kind="ExternalInput")
    coef_c = nc.dram_tensor("coef_c", [128, HL * NTB], F32, kind="ExternalInput")
    bqkv_c = nc.dram_tensor("bqkv_c", [128, 12], F32, kind="ExternalInput")
    bgate_c = nc.dram_tensor("bgate_c", [128, NPAIR], F32, kind="ExternalInput")
    bp_c = nc.dram_tensor("bp_c", [128, HL], F32, kind="ExternalInput")

    y_fm = nc.dram_tensor("y_fm", [D, N], F32, kind="ExternalOutput")
    if dbg:
        d_q2 = nc.dram_tensor("d_q2", [128, N], BF16, kind="ExternalOutput")
        d_k2 = nc.dram_tensor("d_k2", [128, NPADCOLS], BF16,
                              kind="ExternalOutput")
        d_v2 = nc.dram_tensor("d_v2", [128, NPADCOLS], BF16,
                              kind="ExternalOutput")
        d_gate = nc.dram_tensor("d_gate", [128, N], BF16, kind="ExternalOutput")
        d_qkT = nc.dram_tensor("d_qkT", [88, N], BF16, kind="ExternalOutput")
        d_E = nc.dram_tensor("d_E", [128, NCH * HL * NS], BF16,
                             kind="ExternalOutput")
        d_qktm = nc.dram_tensor("d_qktm", [128, NCH * 2 * 88], BF16,
                                kind="ExternalOutput")
        d_ast = nc.dram_tensor("d_ast", [88, N], BF16, kind="ExternalOutput")
        d_ofm = nc.dram_tensor("d_ofm", [128, N], BF16, kind="ExternalOutput")

    CH = NCH * HL               # 128 (c,h) groups
    W = NCH * HL * NTB          # 5632

    with tile.TileContext(nc) as tc, ExitStack() as S:
        # ---- persistent pools ----
        const = S.enter_context(tc.tile_pool(name="const", bufs=1))
        big = S.enter_context(tc.tile_pool(name="big", bufs=1, side="right"))
        k2 = [big.tile([128, NPADCOLS], BF16, tag=f"k2_{p}", name=f"k2_{p}")
              for p in range(NPAIR)]
        v2 = [big.tile([128, NPADCOLS], BF16, tag=f"v2_{p}", name=f"v2_{p}")
              for p in range(NPAIR)]
        gate = [big.tile([128, N], BF16, tag=f"g_{p}", name=f"g_{p}")
                for p in range(NPAIR)]
        out_fm = [big.tile([128, N], BF16, tag=f"o_{p}", name=f"o_{p}")
                  for p in range(NPAIR)]

        S_as = ExitStack()
        arow = S_as.enter_context(tc.tile_pool(name="arow", bufs=1, side="right"))
        A_stage = [arow.tile([88, N], BF16, tag=f"ast{t}", name=f"ast{t}")
                   for t in range(2)]

        S_e = ExitStack()
        ep = S_e.enter_context(tc.tile_pool(name="ep", bufs=1))
        E_tm = ep.tile([128, NCH * HL * NS], BF16)
        SE_tm = ep.tile([128, NCH * HL], F32)
        EC_t = ep.tile([128, W], BF16)
        qk_Tp = S_e.enter_context(tc.tile_pool(name="qkTp", bufs=1, side="right"))
        qk_T = [qk_Tp.tile([88, N], BF16, tag=f"qkT{t}", name=f"qkT{t}")
                for t in range(2)]

        # ======== P1: projections (input DMAs first for fast start) ========
        S_q = ExitStack()
        qp = S_q.enter_context(tc.tile_pool(name="qp", bufs=1, side="right"))
        q2 = [qp.tile([128, N], BF16, tag=f"q2_{p}", name=f"q2_{p}")
              for p in range(NPAIR)]
        bqkv_t = const.tile([128, 12], F32)
        bgate_t = const.tile([128, NPAIR], F32)
        with tc.tile_pool(name="p1x", bufs=1) as p1x, \
             tc.tile_pool(name="p1", bufs=2) as p1, \
             tc.tile_pool(name="p1ps", bufs=4, space="PSUM") as p1ps:
            xr = []
            for kc in range(D // 128):
                xrt = p1x.tile([128, N], BF16, tag=f"xr{kc}", name=f"xr{kc}")
                nc.sync.dma_start(xrt[:], xT[128 * kc:128 * (kc + 1), :])
                xr.append(xrt)
            nc.sync.dma_start(bqkv_t[:], bqkv_c[:])
            nc.sync.dma_start(bgate_t[:], bgate_c[:])
            for p in range(NPAIR):
                nc.vector.memset(k2[p][:, 0:PAD], 0.0)
                nc.vector.memset(v2[p][:, 0:PAD], 0.0)
            for fc in range(16):
                wr = p1.tile([128, 8 * 128], BF16, tag="wr")
                nc.sync.dma_start(
                    wr[:].rearrange("p (a m) -> p a m", a=8),
                    wT[:, 128 * fc:128 * (fc + 1)]
                    .rearrange("(a p) m -> p a m", p=128))
                sect, pair = fc // 4, fc % 4
                for nb in range(NNB):
                    ps = p1ps.tile([128, 512], F32, tag="proj")
                    for kc in range(D // 128):
                        nc.tensor.matmul(
                            ps[:], wr[:, 128 * kc:128 * (kc + 1)],
                            xr[kc][:, 512 * nb:512 * (nb + 1)],
                            start=(kc == 0), stop=(kc == 7))
                    sl = slice(512 * nb, 512 * (nb + 1))
                    slp = slice(PAD + 512 * nb, PAD + 512 * (nb + 1))
                    if sect == 0:
                        nc.scalar.activation(q2[pair][:, sl], ps[:],
                                             AF.Identity, bias=bqkv_t[:, fc:fc + 1])
                    elif sect == 1:
                        nc.scalar.activation(k2[pair][:, slp], ps[:],
                                             AF.Identity, bias=bqkv_t[:, fc:fc + 1])
                    elif sect == 2:
                        nc.scalar.activation(v2[pair][:, slp], ps[:],
                                             AF.Identity, bias=bqkv_t[:, fc:fc + 1])
                    else:
                        nc.scalar.activation(gate[pair][:, sl], ps[:], AF.Sigmoid,
                                             bias=bgate_t[:, pair:pair + 1])

        # ---- remaining constants (issued after P1 inputs) ----
        ident_f = const.tile([128, 128], F32)
        make_identity(nc, ident_f)
        ident_b = const.tile([128, 128], BF16)
        nc.vector.tensor_copy(ident_b[:], ident_f[:])
        ones_f = const.tile([128, 2], F32)
        nc.vector.memset(ones_f[:], 0.0)
        nc.vector.memset(ones_f[0:64, 0:1], 1.0)
        nc.vector.memset(ones_f[64:128, 1:2], 1.0)
        ones2 = const.tile([128, 2], BF16)
        nc.vector.tensor_copy(ones2[:], ones_f[:])
        wqs_r = const.tile([128, 2 * NS], BF16)
        nc.sync.dma_start(wqs_r[:], wqs[:])
        prior_t = const.tile([128, HL * NS], F32)
        nc.sync.dma_start(prior_t[:], prior_c[:])
        bias_t = const.tile([128, HL * NTB], F32)
        nc.sync.dma_start(bias_t[:], bias_c[:])
        coef_t = const.tile([128, HL * NTB], F32)
        nc.sync.dma_start(coef_t[:], coef_c[:])
        bp_t = const.tile([128, HL], F32)
        nc.sync.dma_start(bp_t[:], bp_c[:])
        sel_t = const.tile([128, 2 * NOFF * 128], BF16)
        nc.sync.dma_start(sel_t[:], sel_c[:])

        if dbg:
            nc.sync.dma_start(d_q2[:], q2[0][:])
            nc.sync.dma_start(d_k2[:], k2[0][:])
            nc.sync.dma_start(d_v2[:], v2[0][:])
            nc.sync.dma_start(d_gate[:], gate[0][:])

        # ======== P2: E = exp(q_offset + prior), SE, EC ========
        with tc.tile_pool(name="p2", bufs=2) as p2, \
             tc.tile_pool(name="p2ps", bufs=2, space="PSUM") as p2ps:
            for c in range(NCH):
                ps = p2ps.tile([128, HL * NS], F32, tag="qs")
                for p in range(NPAIR):
                    nc.tensor.matmul(
                        ps[:, 2 * NS * p:2 * NS * (p + 1)],
                        q2[p][:, 128 * c:128 * (c + 1)], wqs_r[:],
                        start=True, stop=True)
                et = p2.tile([128, HL * NS], F32, tag="et")
                nc.vector.tensor_tensor(out=et[:], in0=ps[:], in1=prior_t[:],
                                        op=ALU.add)
                nc.scalar.activation(E_tm[:, HL * NS * c:HL * NS * (c + 1)],
                                     et[:], AF.Exp)
            nc.vector.tensor_reduce(
                SE_tm[:].rearrange("p (c h) -> p c h", h=HL).unsqueeze(-1),
                E_tm[:].rearrange("p (c h s) -> p c h s", h=HL, s=NS),
                axis=AX.X, op=ALU.add)
        # EC = E (bcast tau) * coef — off the P5 critical path
        nc.gpsimd.tensor_copy(
            EC_t[:].rearrange("p (ch s f) -> p ch s f", s=NS, f=4),
            E_tm[:].rearrange("p (ch s) -> p ch s", s=NS)
            .unsqueeze(-1).broadcast_to([128, CH, NS, 4]))
        nc.vector.tensor_tensor(
            out=EC_t[:].rearrange("p (c w) -> p c w", w=HL * NTB),
            in0=EC_t[:].rearrange("p (c w) -> p c w", w=HL * NTB),
            in1=coef_t[:].unsqueeze(1).broadcast_to([128, NCH, HL * NTB]),
            op=ALU.mult)

        # ======== P3: qk scores -> qk_T ========
        with tc.tile_pool(name="p3", bufs=2) as p3, \
             tc.tile_pool(name="p3s", bufs=2) as p3s, \
             tc.tile_pool(name="p3ps", bufs=2, space="PSUM") as p3ps:
            for oi, off in enumerate(OFFSETS):
                prods = []
                for p in range(NPAIR):
                    prod = p3.tile([128, N], BF16, tag=f"prod{p}")
                    nc.vector.tensor_tensor(
                        out=prod[:], in0=q2[p][:],
                        in1=k2[p][:, PAD - off:PAD - off + N], op=ALU.mult)
                    prods.append(prod)
                ps = p3ps.tile([128, N], F32, tag="qkps")
                for nb in range(NNB):
                    for p in range(NPAIR):
                        nc.tensor.matmul(
                            ps[32 * p:32 * p + 2, 512 * nb:512 * (nb + 1)],
                            ones2[:], prods[p][:, 512 * nb:512 * (nb + 1)],
                            start=True, stop=True,
                            tile_position=(0, 32 * p))
                st = p3s.tile([128, N], BF16, tag="stage")
                nc.scalar.copy(st[:], ps[:])
                for t in range(2):
                    for ph in range(2):
                        p = 2 * t + ph
                        dst = qk_T[t][:].rearrange(
                            "(a j o) n -> a j o n", a=2, j=2)[ph, :, oi, :]
                        nc.sync.dma_start(dst, st[32 * p:32 * p + 2, :])
        S_q.close()  # q2 released
        if dbg:
            nc.sync.dma_start(d_qkT[:], qk_T[0][:])
            nc.sync.dma_start(d_E[:], E_tm[:])

        # ======== P4: transpose qk_T -> token-major ========
        qk_tm = ep.tile([128, NCH * 2 * 88], BF16, name="qk_tm")
        with tc.tile_pool(name="p4ps", bufs=4, space="PSUM") as p4ps:
            for c in range(NCH):
                for t in range(2):
                    ps = p4ps.tile([128, 88], BF16, tag="tp")
                    nc.tensor.transpose(
                        ps[:], qk_T[t][:, 128 * c:128 * (c + 1)],
                        ident_b[0:88, 0:88])
                    nc.scalar.copy(
                        qk_tm[:, 176 * c + 88 * t:176 * c + 88 * (t + 1)],
                        ps[:])
        if dbg:
            nc.sync.dma_start(d_qktm[:], qk_tm[:])

        # ======== P5..P8 pipelined by token halves / nb blocks ========
        with tc.tile_pool(name="p5", bufs=2) as p5, \
             tc.tile_pool(name="p6ps", bufs=2, space="PSUM") as p6ps, \
             tc.tile_pool(name="p7", bufs=6) as p7, \
             tc.tile_pool(name="p7ps", bufs=3, space="PSUM") as p7ps, \
             tc.tile_pool(name="p7po", bufs=2, space="PSUM") as p7po, \
             tc.tile_pool(name="p8", bufs=2) as p8, \
             tc.tile_pool(name="p8g", bufs=1) as p8g, \
             tc.tile_pool(name="p8ps", bufs=1, space="PSUM") as p8ps:
            wo_r = []
            for p in range(NPAIR):
                wor = p8g.tile([128, D], BF16, tag=f"wor{p}", name=f"wor{p}")
                nc.sync.dma_start(wor[:], woT[128 * p:128 * (p + 1), :])
                wo_r.append(wor)
            sel_v = sel_t[:].rearrange("p (i m) -> p i m", i=2 * NOFF)

            def p5_half(hf):
                c0, c1 = 8 * hf, 8 * hf + 8   # chunk range
                ncc = c1 - c0                 # 8 chunks
                Wh = ncc * HL * NTB           # 2816
                CHh = ncc * HL
                qk_s = qk_tm[:, 176 * c0:176 * c1]
                qk8 = qk_s.rearrange("p (ch o) -> p ch o", o=NOFF)
                qk9 = qk_s.rearrange("p (ch e two) -> p ch e two", e=NS, two=2)
                EC_s = EC_t[:, HL * NTB * c0:HL * NTB * c1]
                SE_s = SE_tm[:, HL * c0:HL * c1]

                y_t = p5.tile([128, Wh], BF16, tag="y")
                y7 = y_t[:].rearrange("p (ch j f) -> p ch j f", j=NS, f=4)
                nc.gpsimd.tensor_copy(
                    y7[:, :, :, 0:1],
                    qk8[:, :, 0:1].unsqueeze(2).broadcast_to([128, CHh, NS, 1]))
                nc.gpsimd.tensor_copy(y7[:, :, 0:1, 1:2],
                                      qk8[:, :, 1:2].unsqueeze(2))
                nc.gpsimd.tensor_copy(y7[:, :, 1:2, 1:2],
                                      qk8[:, :, 2:3].unsqueeze(2))
                nc.gpsimd.tensor_copy(y7[:, :, 2:11, 1:2],
                                      qk9[:, :, 2:11, 0:1])
                nc.gpsimd.tensor_copy(y7[:, :, 0:1, 2:3],
                                      qk8[:, :, 2:3].unsqueeze(2))
                nc.gpsimd.tensor_copy(y7[:, :, 1:10, 2:3],
                                      qk9[:, :, 2:11, 0:1])
                nc.gpsimd.tensor_copy(y7[:, :, 10:11, 2:3],
                                      qk9[:, :, 10:11, 0:1])
                nc.gpsimd.tensor_copy(y7[:, :, 0:10, 3:4],
                                      qk9[:, :, 1:11, 1:2])
                nc.gpsimd.tensor_copy(y7[:, :, 10:11, 3:4],
                                      qk9[:, :, 10:11, 1:2])

                nc.vector.tensor_tensor(
                    out=y_t[:].rearrange("p (c w) -> p c w", w=HL * NTB),
                    in0=y_t[:].rearrange("p (c w) -> p c w", w=HL * NTB),
                    in1=bias_t[:].unsqueeze(1).broadcast_to([128, ncc, HL * NTB]),
                    op=ALU.add)
                m0 = p5.tile([128, Wh], BF16, tag="m0")
                nc.vector.tensor_scalar(out=m0[:], in0=y_t[:], scalar1=0.0,
                                        scalar2=None, op0=ALU.min)
                nc.scalar.activation(m0[:], m0[:], AF.Exp)
                nc.vector.tensor_scalar(out=y_t[:], in0=y_t[:], scalar1=0.0,
                                        scalar2=None, op0=ALU.max)
                nc.vector.tensor_tensor(out=m0[:], in0=m0[:], in1=y_t[:],
                                        op=ALU.add)
                nc.vector.tensor_tensor(out=m0[:], in0=m0[:], in1=EC_s,
                                        op=ALU.mult)

                ab = p5.tile([128, CHh], F32, tag="ab")
                t0 = p5.tile([128, CHh], F32, tag="t0")
                nc.vector.tensor_scalar(out=ab[:], in0=qk8[:, :, 0:1].squeeze(-1),
                                        scalar1=0.0, scalar2=None, op0=ALU.min)
                nc.scalar.activation(ab[:], ab[:], AF.Exp)
                nc.vector.tensor_scalar(out=t0[:], in0=qk8[:, :, 0:1].squeeze(-1),
                                        scalar1=0.0, scalar2=None, op0=ALU.max)
                nc.vector.tensor_tensor(out=ab[:], in0=ab[:], in1=t0[:],
                                        op=ALU.add)
                nc.vector.tensor_tensor(out=ab[:], in0=ab[:], in1=SE_s,
                                        op=ALU.mult)
                nc.vector.tensor_tensor(
                    out=ab[:].rearrange("p (c h) -> p c h", h=HL),
                    in0=ab[:].rearrange("p (c h) -> p c h", h=HL),
                    in1=bp_t[:].unsqueeze(1).broadcast_to([128, ncc, HL]),
                    op=ALU.mult)

                z_t = p5.tile([128, CHh], F32, tag="z")
                nc.vector.tensor_reduce(
                    z_t[:].rearrange("p (c h) -> p c h", h=HL).unsqueeze(-1),
                    m0[:].rearrange("p (c h t) -> p c h t", h=HL, t=NTB),
                    axis=AX.X, op=ALU.add, apply_absolute_value=True)
                nc.vector.tensor_tensor(out=z_t[:], in0=z_t[:], in1=ab[:],
                                        op=ALU.add)
                nc.vector.scalar_tensor_tensor(
                    out=z_t[:], in0=SE_s, scalar=1e-6, in1=z_t[:],
                    op0=ALU.mult, op1=ALU.add)
                nc.vector.reciprocal(z_t[:], z_t[:])
                nc.vector.tensor_tensor(
                    out=m0[:].rearrange("p (ch t) -> p ch t", t=NTB),
                    in0=m0[:].rearrange("p (ch t) -> p ch t", t=NTB),
                    in1=z_t[:].unsqueeze(-1).broadcast_to([128, CHh, NTB]),
                    op=ALU.mult)
                nc.vector.tensor_tensor(out=ab[:], in0=ab[:], in1=z_t[:],
                                        op=ALU.mult)

                A_tm = p5.tile([128, ncc * HL * NOFF], F32, tag="atm")
                A8 = A_tm[:].rearrange("p (ch o) -> p ch o", o=NOFF)
                A9 = A_tm[:].rearrange("p (ch e two) -> p ch e two", e=NS, two=2)
                m7 = m0[:].rearrange("p (ch j f) -> p ch j f", j=NS, f=4)
                nc.vector.tensor_reduce(A8[:, :, 0:1].unsqueeze(-1),
                                        m7[:, :, :, 0:1].transpose([0, 1, 3, 2]),
                                        axis=AX.X, op=ALU.add)
                nc.vector.tensor_tensor(out=A8[:, :, 0:1].squeeze(-1),
                                        in0=A8[:, :, 0:1].squeeze(-1),
                                        in1=ab[:], op=ALU.add)
                nc.vector.tensor_copy(A8[:, :, 1:2],
                                      m7[:, :, 0:1, 1:2].squeeze(-1))
                nc.vector.tensor_tensor(out=A9[:, :, 1:11, 0:1],
                                        in0=m7[:, :, 1:11, 1:2],
                                        in1=m7[:, :, 0:10, 2:3], op=ALU.add)
                nc.vector.tensor_copy(A9[:, :, 1:11, 1:2],
                                      m7[:, :, 0:10, 3:4])

                # P6: transpose to A_stage columns of this half
                for ci in range(ncc):
                    c = c0 + ci
                    for t in range(2):
                        ps2 = p6ps.tile([88, 128], F32, tag="tpb")
                        nc.tensor.transpose(
                            ps2[:],
                            A_tm[:, 176 * ci + 88 * t:176 * ci + 88 * (t + 1)],
                            ident_f[:])
                        nc.scalar.copy(A_stage[t][:, 128 * c:128 * (c + 1)],
                                       ps2[:])

            def p7_block(nb):
                n0 = 512 * nb
                for p in range(NPAIR):
                    t, ph = p // 2, p % 2
                    po = p7po.tile([128, 512], F32, tag="avo")
                    valid = [(oi, off) for oi, off in enumerate(OFFSETS)
                             if off < n0 + 512]
                    for vi, (oi, off) in enumerate(valid):
                        pa = p7ps.tile([128, 512], F32, tag="aexp")
                        nc.tensor.matmul(
                            pa[:], sel_v[0:88, NOFF * ph + oi, :],
                            A_stage[t][:, n0:n0 + 512],
                            start=True, stop=True)
                        tmp = p7.tile([128, 512], BF16, tag="avt")
                        nc.vector.tensor_tensor(
                            out=tmp[:],
                            in0=v2[p][:, PAD + n0 - off:PAD + n0 + 512 - off],
                            in1=pa[:], op=ALU.mult)
                        nc.tensor.matmul(
                            po[:], ident_b[:], tmp[:],
                            start=(vi == 0), stop=(vi == len(valid) - 1))
                    nc.scalar.copy(out_fm[p][:, n0:n0 + 512], po[:])

            def p8_block(nb):
                n0 = 512 * nb
                for p in range(NPAIR):
                    nc.gpsimd.tensor_tensor(
                        out=out_fm[p][:, n0:n0 + 512],
                        in0=out_fm[p][:, n0:n0 + 512],
                        in1=gate[p][:, n0:n0 + 512], op=ALU.mult)
                for dc in range(D // 128):
                    ps = p8ps.tile([128, 512], F32, tag="yps")
                    for p in range(NPAIR):
                        nc.tensor.matmul(
                            ps[:], wo_r[p][:, 128 * dc:128 * (dc + 1)],
                            out_fm[p][:, n0:n0 + 512],
                            start=(p == 0), stop=(p == NPAIR - 1))
                    yt = p8.tile([128, 512], F32, tag="yt")
                    nc.scalar.copy(yt[:], ps[:])
                    nc.sync.dma_start(
                        y_fm[128 * dc:128 * (dc + 1), n0:n0 + 512], yt[:])

            p5_half(0)
            p7_block(0)
            p8_block(0)
            p5_half(1)
            if dbg:
                nc.sync.dma_start(d_ast[:], A_stage[0][:])
            p7_block(1)
            p8_block(1)
            p7_block(2)
            p8_block(2)
            p7_block(3)
            p8_block(3)
            if dbg:
                nc.sync.dma_start(d_ofm[:], out_fm[0][:])

        S_e.close()
        S_as.close()
    nc.compile()
    return nc


# ===========================================================================
# host side
# ===========================================================================

_SEL = np.zeros((128, 2 * NOFF * 128), np.float32)
for _ph in range(2):
    for _oi in range(NOFF):
        _i = NOFF * _ph + _oi
        _SEL[44 * _ph + _oi, 128 * _i:128 * _i + 64] = 1.0
        _SEL[44 * _ph + NOFF + _oi, 128 * _i + 64:128 * (_i + 1)] = 1.0


def _bf16(a):
    return np.ascontiguousarray(a).astype(ml_dtypes.bfloat16)


def _make_inputs(x, W_qkv, b_qkv, W_out, W_gate, b_gate, scale_gain, W_qscale,
                 identity_bypass, pos_bias, b, g):
    hg0 = g * HL
    rows = slice(hg0 * HD, (hg0 + HL) * HD)
    Wq = W_qkv[0 * D:1 * D][rows]
    Wk = W_qkv[1 * D:2 * D][rows]
    Wv = W_qkv[2 * D:3 * D][rows]
    Wg = W_gate[rows]
    wTv = np.concatenate([Wq, Wk, Wv, Wg], axis=0).T.copy()
    woTv = W_out[:, rows].T.copy()

    wqsv = np.zeros((128, 2 * NS), np.float32)
    wqsv[0:64, 0:NS] = W_qscale.T
    wqsv[64:128, NS:2 * NS] = W_qscale.T

    prior = np.zeros((HL, NS), np.float32)
    for h in range(HL):
        prior[h] = scale_gain[:, hg0 + h]
    prior_v = np.broadcast_to(prior.reshape(1, -1), (128, HL * NS)).copy()

    bias = np.zeros((HL, NTB), np.float32)
    coef = np.zeros((HL, NTB), np.float32)
    for h in range(HL):
        for (j, tau, off, full_idx) in TAPS:
            bias[h, full_idx] = pos_bias[full_idx, hg0 + h]
            coef[h, full_idx] = D4[tau]
    bias_v = np.broadcast_to(bias.reshape(1, -1), (128, HL * NTB)).copy()
    coef_v = np.broadcast_to(coef.reshape(1, -1), (128, HL * NTB)).copy()

    bqkv = np.zeros((128, 12), np.float32)
    for sect, bb in enumerate([b_qkv[0:D], b_qkv[D:2 * D], b_qkv[2 * D:3 * D]]):
        sl = bb[rows]
        for pair in range(NPAIR):
            bqkv[:, sect * 4 + pair] = sl[128 * pair:128 * (pair + 1)]
    bgate_v = np.zeros((128, NPAIR), np.float32)
    gsl = b_gate[rows]
    for pair in range(NPAIR):
        bgate_v[:, pair] = gsl[128 * pair:128 * (pair + 1)]

    bp = np.log1p(np.exp(identity_bypass[hg0:hg0 + HL])).astype(np.float32)
    bp_v = np.broadcast_to(bp.reshape(1, -1), (128, HL)).copy()

    return {
        "xT": _bf16(x[b].T),
        "wT": _bf16(wTv),
        "woT": _bf16(woTv),
        "wqs": _bf16(wqsv),
        "sel_c": _bf16(_SEL),
        "prior_c": np.ascontiguousarray(prior_v),
        "bias_c": np.ascontiguousarray(bias_v),
        "coef_c": np.ascontiguousarray(coef_v),
        "bqkv_c": bqkv,
        "bgate_c": bgate_v,
        "bp_c": np.ascontiguousarray(bp_v),
    }


def kernel(x, W_qkv, b_qkv, W_out, b_out, W_gate, b_gate, scale_gain, W_qscale,
           identity_bypass, pos_bias):
    x = np.asarray(x, np.float32)
    args = [np.asarray(a, np.float32) for a in
            (W_qkv, b_qkv, W_out, W_gate, b_gate, scale_gain, W_qscale,
             identity_bypass, pos_bias)]
    (W_qkv, b_qkv, W_out, W_gate, b_gate, scale_gain, W_qscale,
     identity_bypass, pos_bias) = args

    if "nc" not in _KERNEL_CACHE:
        _KERNEL_CACHE["nc"] = build_kernel()
    nc = _KERNEL_CACHE["nc"]

    in_maps = []
    for core in range(8):
        b, g = core % 4, core // 4
        in_maps.append(_make_inputs(x, W_qkv, b_qkv, W_out, W_gate, b_gate,
                                    scale_gain, W_qscale, identity_bypass,
                                    pos_bias, b, g))
    res = run_bass_kernel_spmd(nc, in_maps, list(range(8)))

    out = np.zeros((B, N, D), np.float32)
    for core in range(8):
        b = core % 4
        out[b] += res.results[core]["y_fm"].T
    out += np.asarray(b_out, np.float32)
    return out
